# revision 9
# baseline (speedup 1.0000x reference)
"""DBRX MoE experts kernel for 8 Trainium2 NeuronCores.

Strategy (expert-parallel with host-side token dispatch):
  - Host computes the (cheap) router: softmax over 16 experts, top-4,
    renormalized gates.  Tokens are gathered per expert.
  - Each core gets NG=2 expert "groups" (16 experts / 8 cores).  Experts are
    sorted by token count: the 8 largest go in group 0, the 8 smallest in
    group 1, and each group's tokens are packed into MG tiles of T_g tokens
    (zero padded, T sized per group to the largest expert in it).  The
    expert's weights are loaded once per group and reused across its tiles.
  - Device (SPMD, one program on all 8 cores) runs the expert FFN:
    h = wsT.T @ x (both halves), act = silu(h1)*h2, y = w2T.T @ act.
    All matmuls in bfloat16 (1 cycle/row, FWL weight loads at 2 elem/cycle,
    half the DMA bytes of fp32; end-to-end ~4e-3 rel err vs 2e-2 budget).
  - Host applies gates and scatter-adds item outputs into the [T, D] output.
    Only the FFN (97% of the FLOPs) runs on device; the dense 16-expert
    reference computation is avoided entirely (4x FLOP saving via top-4).

Self-contained: hardcodes T=4096 tokens, D=1024, I=2048, E=16, top_k=4,
8 cores.
"""

import sys

if "/opt/trn_rl_repo" not in sys.path:
    sys.path.insert(0, "/opt/trn_rl_repo")

import ml_dtypes
import numpy as np

import concourse.bacc as bacc
import concourse.mybir as mybir
import concourse.tile as tile
from concourse.bass_utils import run_bass_kernel_spmd

TOP_K = 4
N_CORES = 8
D = 1024
I = 2048
E = 16
DC = D // 128  # 8 contraction chunks for mm1 / output blocks for mm2
IC = I // 128  # 16 intermediate blocks
CB = 2 * I // 128  # 32 column blocks of ws

BF16 = ml_dtypes.bfloat16

TRACE = False
LAST_EXEC_NS = None

_compiled = {}  # shapes tuple -> nc


def _build_program(shapes):
    """shapes: tuple of (MG, T) per group (one group = one expert)."""
    bf16 = mybir.dt.bfloat16
    f32 = mybir.dt.float32
    NG = len(shapes)
    nc = bacc.Bacc("TRN2", target_bir_lowering=False, debug=False, num_devices=N_CORES)

    xTs, yTs = [], []
    for g, (MG, T) in enumerate(shapes):
        xTs.append(
            nc.dram_tensor(f"xT{g}", [MG, 128, DC, T], bf16, kind="ExternalInput")
        )
        yTs.append(
            nc.dram_tensor(f"yT{g}", [MG, DC, 128, T], bf16, kind="ExternalOutput")
        )
    wsT = nc.dram_tensor("wsT", [NG, CB, 128, DC, 128], bf16, kind="ExternalInput")
    w2T = nc.dram_tensor("w2T", [NG, DC, 128, IC, 128], bf16, kind="ExternalInput")

    with tile.TileContext(nc) as tc:
        with (
            tc.tile_pool(name="xp", bufs=1) as xp,
            tc.tile_pool(name="wp", bufs=4) as wp,
            tc.tile_pool(name="w2p", bufs=3) as w2p,
            tc.tile_pool(name="actp", bufs=1) as actp,
            tc.tile_pool(name="sp", bufs=3) as sp,
            tc.tile_pool(name="pp", bufs=6, space="PSUM") as pp,
            tc.tile_pool(name="pp2", bufs=2, space="PSUM") as pp2,
        ):
            for g, (MG, T) in enumerate(shapes):
                # Startup-critical transfers split across the two HWDGE issue
                # engines (sync/scalar): w1t0 halves first, then the j=0 x
                # chunks, then j>0, then v1t0 (first needed only after the 8
                # ps1 matmuls).
                w1t0 = wp.tile([128, DC, 128], bf16, tag="ws", name=f"w1t0_{g}")
                nc.sync.dma_start(w1t0[:, : DC // 2], wsT.ap()[g, 0, :, : DC // 2])
                nc.scalar.dma_start(
                    w1t0[:, DC // 2 :], wsT.ap()[g, 0, :, DC // 2 :]
                )
                xts = []
                acts = []
                for j in range(MG):
                    # exact-size per-group tiles: contiguous per-partition
                    # destination, one DMA descriptor per partition line
                    xt = xp.tile(
                        [128, DC, T], bf16, tag=f"x{g}", bufs=MG, name=f"x_{g}_{j}"
                    )
                    xts.append(xt)
                    act = actp.tile(
                        [128, IC, T], bf16, tag=f"act{g}", bufs=MG,
                        name=f"act_{g}_{j}",
                    )
                    acts.append(act)
                for j in range(MG):
                    for k, dc in enumerate(range(0, DC, 2)):
                        eng = nc.scalar if (j + k) % 2 else nc.sync
                        eng.dma_start(
                            xts[j][:, dc : dc + 2], xTs[g].ap()[j, :, dc : dc + 2]
                        )
                    if j == 0:
                        v1t0 = wp.tile(
                            [128, DC, 128], bf16, tag="ws", name=f"v1t0_{g}"
                        )
                        nc.sync.dma_start(
                            v1t0[:, : DC // 2], wsT.ap()[g, IC, :, : DC // 2]
                        )
                        nc.scalar.dma_start(
                            v1t0[:, DC // 2 :], wsT.ap()[g, IC, :, DC // 2 :]
                        )

                # mm1 + SwiGLU: weights outer, token tiles inner (weight reuse)
                for ip in range(IC):
                    if ip == 0:
                        w1t, v1t = w1t0, v1t0
                    else:
                        w1t = wp.tile([128, DC, 128], bf16, tag="ws")
                        nc.sync.dma_start(w1t[:], wsT.ap()[g, ip])
                        v1t = wp.tile([128, DC, 128], bf16, tag="ws")
                        nc.sync.dma_start(v1t[:], wsT.ap()[g, IC + ip])
                    for j in range(MG):
                        ps1 = pp.tile([128, T], f32, tag="h")
                        ps2 = pp.tile([128, T], f32, tag="h")
                        for dc in range(DC):
                            nc.tensor.matmul(
                                ps1[:], w1t[:, dc], xts[j][:, dc],
                                start=(dc == 0), stop=(dc == DC - 1),
                            )
                        for dc in range(DC):
                            nc.tensor.matmul(
                                ps2[:], v1t[:, dc], xts[j][:, dc],
                                start=(dc == 0), stop=(dc == DC - 1),
                            )
                        st = sp.tile([128, T], f32, tag="silu")
                        nc.scalar.activation(
                            st[:], ps1[:], mybir.ActivationFunctionType.Silu
                        )
                        nc.vector.tensor_mul(acts[j][:, ip], st[:], ps2[:])

                # mm2: w2 slabs outer, token tiles inner (weight reuse)
                last_g = g == len(shapes) - 1
                for db in range(DC):
                    w2t = w2p.tile([128, IC, 128], bf16, tag="w2")
                    nc.sync.dma_start(w2t[:], w2T.ap()[g, db])
                    for j in range(MG):
                        # last slab of the kernel: compute/copy/DMA in column
                        # halves (separate PSUM tiles — same tile would add a
                        # false matmul->copy dependency) so the writeback
                        # overlaps the final matmuls
                        halves = (
                            2 if last_g and db == DC - 1 and j == MG - 1 else 1
                        )
                        Th = T // halves
                        for h in range(halves):
                            sl = slice(h * Th, (h + 1) * Th)
                            ps3 = pp2.tile([128, Th], f32, tag="y")
                            for ic in range(IC):
                                nc.tensor.matmul(
                                    ps3[:], w2t[:, ic], acts[j][:, ic, sl],
                                    start=(ic == 0), stop=(ic == IC - 1),
                                )
                            ot = sp.tile([128, Th], bf16, tag="yout")
                            nc.any.tensor_copy(ot[:], ps3[:])
                            eng = nc.scalar if h % 2 else nc.sync
                            eng.dma_start(yTs[g].ap()[j, db, :, sl], ot[:])
    nc.compile()
    return nc


def _routing(x, rw):
    logits = x @ rw.T
    m = logits.max(-1, keepdims=True)
    p = np.exp(logits - m)
    p /= p.sum(-1, keepdims=True)
    topk_idx = np.argpartition(-p, TOP_K - 1, axis=-1)[:, :TOP_K]
    topk_val = np.take_along_axis(p, topk_idx, -1)
    topk_val = topk_val / topk_val.sum(-1, keepdims=True)
    return topk_idx, topk_val


def _group_shape(cmax):
    """Pick (MG, T) so MG*T >= cmax, T in [128, 512] mult of 4 (even halves),
    minimizing MG*T (tie: prefer fewer/larger tiles)."""
    best = None
    for MG in range(1, 17):
        T = -(-cmax // MG) if cmax else 128
        T = (T + 3) // 4 * 4
        if T > 512:
            continue
        T = max(T, 128)
        if best is None or MG * T < best[0]:
            best = (MG * T, MG, T)
    assert best is not None
    return best[1], best[2]


def _tile_ws(ws_e):
    # [cb, p, dc, col] = ws_e[cb*128+col, dc*128+p]
    return np.ascontiguousarray(
        ws_e.reshape(CB, 128, DC, 128).transpose(0, 3, 2, 1)
    )


def _tile_w2(w2_e):
    # [db, p, ic, col] = w2_e[db*128+col, ic*128+p]
    return np.ascontiguousarray(
        w2_e.reshape(DC, 128, IC, 128).transpose(0, 3, 2, 1)
    )


NGROUPS = 4  # expert pieces per core: splits experts for load balance


def _plan(counts, n_groups):
    """Split the E experts' token counts into n_groups*N_CORES near-equal
    pieces (each piece = contiguous token range of one expert), sort pieces
    by size, and pack rank-consecutive pieces into groups of N_CORES slots.
    Returns (shapes, groups): shapes[g] = (MG, T); groups[g][c] =
    (expert, start, size) for core c."""
    nslots = n_groups * N_CORES
    n = np.ones(E, dtype=np.int64)
    while n.sum() < nslots:
        piece = -(-counts // n)
        n[int(np.argmax(piece))] += 1
    pieces = []
    for e in range(E):
        k = int(n[e])
        base, rem = divmod(int(counts[e]), k)
        off = 0
        for i in range(k):
            sz = base + (1 if i < rem else 0)
            pieces.append((sz, e, off))
            off += sz
    pieces.sort(key=lambda p: (-p[0], p[1], p[2]))
    groups = [
        [(e, off, sz) for sz, e, off in pieces[g * N_CORES : (g + 1) * N_CORES]]
        for g in range(n_groups)
    ]
    shapes = tuple(
        _group_shape(max(sz for _, _, sz in grp)) for grp in groups
    )
    return shapes, groups


def kernel(hidden_states, router_w, ws, w2s):
    global LAST_EXEC_NS
    x = np.ascontiguousarray(np.asarray(hidden_states, dtype=np.float32))
    rw = np.asarray(router_w, dtype=np.float32)
    ws = np.asarray(ws, dtype=np.float32)
    w2s = np.asarray(w2s, dtype=np.float32)
    T_tok = x.shape[0]

    topk_idx, topk_val = _routing(x, rw)

    expert_tok = []
    expert_gate = []
    for e in range(E):
        hit = topk_idx == e
        rows = np.nonzero(hit.any(-1))[0]
        gv = np.where(hit[rows], topk_val[rows], 0.0).sum(-1).astype(np.float32)
        expert_tok.append(rows)
        expert_gate.append(gv)

    counts = np.array([len(t) for t in expert_tok])
    shapes, groups = _plan(counts, NGROUPS)
    NG = len(shapes)

    if shapes not in _compiled:
        _compiled[shapes] = _build_program(shapes)
    nc = _compiled[shapes]

    x16 = x.astype(BF16)
    ws16 = {}
    w2s16 = {}
    in_maps = []
    for c in range(N_CORES):
        m = {}
        wsT_b = np.empty((NG, CB, 128, DC, 128), dtype=BF16)
        w2T_b = np.empty((NG, DC, 128, IC, 128), dtype=BF16)
        for g, (MG, T) in enumerate(shapes):
            e, off, sz = groups[g][c]
            if e not in ws16:
                ws16[e] = _tile_ws(ws[e].astype(BF16))
                w2s16[e] = _tile_w2(w2s[e].astype(BF16))
            wsT_b[g] = ws16[e]
            w2T_b[g] = w2s16[e]
            xT_b = np.zeros((MG, 128, DC, T), dtype=BF16)
            toks = expert_tok[e][off : off + sz]
            for j in range(MG):
                seg = toks[j * T : (j + 1) * T]
                nn = len(seg)
                if nn == 0:
                    continue
                xT_b[j, :, :, :nn] = x16[seg].reshape(nn, DC, 128).transpose(2, 1, 0)
            m[f"xT{g}"] = xT_b
        m["wsT"] = wsT_b
        m["w2T"] = w2T_b
        in_maps.append(m)

    res = run_bass_kernel_spmd(
        nc, in_maps, core_ids=list(range(N_CORES)), trace=TRACE
    )
    LAST_EXEC_NS = res.exec_time_ns

    out = np.zeros((T_tok, D), dtype=np.float32)
    for g, (MG, T) in enumerate(shapes):
        for c in range(N_CORES):
            e, off, sz = groups[g][c]
            toks = expert_tok[e][off : off + sz]
            gates = expert_gate[e][off : off + sz]
            yT_c = np.asarray(res.results[c][f"yT{g}"]).astype(np.float32)
            for j in range(MG):
                seg = toks[j * T : (j + 1) * T]
                nn = len(seg)
                if nn == 0:
                    break
                y_item = yT_c[j].transpose(2, 0, 1).reshape(T, D)[:nn]
                out[seg] += gates[j * T : (j + 1) * T][:, None] * y_item
    return out


# revision 10
# speedup vs baseline: 1.0158x; 1.0158x over previous
"""DBRX MoE experts kernel for 8 Trainium2 NeuronCores.

Strategy (expert-parallel with host-side token dispatch):
  - Host computes the (cheap) router: softmax over 16 experts, top-4,
    renormalized gates.  Tokens are gathered per expert.
  - Each core gets NG=2 expert "groups" (16 experts / 8 cores).  Experts are
    sorted by token count: the 8 largest go in group 0, the 8 smallest in
    group 1, and each group's tokens are packed into MG tiles of T_g tokens
    (zero padded, T sized per group to the largest expert in it).  The
    expert's weights are loaded once per group and reused across its tiles.
  - Device (SPMD, one program on all 8 cores) runs the expert FFN:
    h = wsT.T @ x (both halves), act = silu(h1)*h2, y = w2T.T @ act.
    All matmuls in bfloat16 (1 cycle/row, FWL weight loads at 2 elem/cycle,
    half the DMA bytes of fp32; end-to-end ~4e-3 rel err vs 2e-2 budget).
  - Host applies gates and scatter-adds item outputs into the [T, D] output.
    Only the FFN (97% of the FLOPs) runs on device; the dense 16-expert
    reference computation is avoided entirely (4x FLOP saving via top-4).

Self-contained: hardcodes T=4096 tokens, D=1024, I=2048, E=16, top_k=4,
8 cores.
"""

import sys

if "/opt/trn_rl_repo" not in sys.path:
    sys.path.insert(0, "/opt/trn_rl_repo")

import ml_dtypes
import numpy as np

import concourse.bacc as bacc
import concourse.mybir as mybir
import concourse.tile as tile
from concourse.bass_utils import run_bass_kernel_spmd

TOP_K = 4
N_CORES = 8
D = 1024
I = 2048
E = 16
DC = D // 128  # 8 contraction chunks for mm1 / output blocks for mm2
IC = I // 128  # 16 intermediate blocks
CB = 2 * I // 128  # 32 column blocks of ws

BF16 = ml_dtypes.bfloat16

TRACE = False
LAST_EXEC_NS = None

_compiled = {}  # shapes tuple -> nc


def _build_program(shapes):
    """shapes: tuple of (MG, T) per group (one group = one expert)."""
    bf16 = mybir.dt.bfloat16
    f32 = mybir.dt.float32
    NG = len(shapes)
    nc = bacc.Bacc("TRN2", target_bir_lowering=False, debug=False, num_devices=N_CORES)

    xTs, yTs = [], []
    for g, (MG, T) in enumerate(shapes):
        xTs.append(
            nc.dram_tensor(f"xT{g}", [MG, 128, DC, T], bf16, kind="ExternalInput")
        )
        yTs.append(
            nc.dram_tensor(f"yT{g}", [MG, DC, 128, T], bf16, kind="ExternalOutput")
        )
    wsT = nc.dram_tensor("wsT", [NG, CB, 128, DC, 128], bf16, kind="ExternalInput")
    w2T = nc.dram_tensor("w2T", [NG, DC, 128, IC, 128], bf16, kind="ExternalInput")

    with tile.TileContext(nc) as tc:
        with (
            tc.tile_pool(name="xp", bufs=max(mg for mg, _ in shapes)) as xp,
            tc.tile_pool(name="wp", bufs=4) as wp,
            tc.tile_pool(name="w2p", bufs=3) as w2p,
            tc.tile_pool(name="actp", bufs=max(mg for mg, _ in shapes)) as actp,
            tc.tile_pool(name="sp", bufs=3) as sp,
            tc.tile_pool(name="pp", bufs=6, space="PSUM") as pp,
            tc.tile_pool(name="pp2", bufs=2, space="PSUM") as pp2,
        ):
            Tmax = max(t for _, t in shapes)
            for g, (MG, T) in enumerate(shapes):
                # Startup-critical transfers split across the two HWDGE issue
                # engines (sync/scalar): w1t0 halves first, then the j=0 x
                # chunks, then v1t0 (first needed only after the 8 ps1
                # matmuls), then the j>0 tiles.
                w1t0 = wp.tile([128, DC, 128], bf16, tag="ws", name=f"w1t0_{g}")
                nc.sync.dma_start(w1t0[:, : DC // 2], wsT.ap()[g, 0, :, : DC // 2])
                nc.scalar.dma_start(
                    w1t0[:, DC // 2 :], wsT.ap()[g, 0, :, DC // 2 :]
                )
                xts = []
                acts = []
                for j in range(MG):
                    # allocate at Tmax with a shared tag (slot reuse across
                    # groups — the reuse dependency paces later groups' x
                    # prefetch behind the weight stream), slice to this T
                    xt = xp.tile(
                        [128, DC, Tmax], bf16, tag="x", name=f"x_{g}_{j}"
                    )[:, :, :T]
                    xts.append(xt)
                    act = actp.tile(
                        [128, IC, Tmax], bf16, tag="act", name=f"act_{g}_{j}"
                    )[:, :, :T]
                    acts.append(act)
                for k, dc in enumerate(range(0, DC, 2)):
                    eng = nc.scalar if k % 2 else nc.sync
                    eng.dma_start(
                        xts[0][:, dc : dc + 2], xTs[g].ap()[0, :, dc : dc + 2]
                    )
                v1t0 = wp.tile([128, DC, 128], bf16, tag="ws", name=f"v1t0_{g}")
                nc.sync.dma_start(v1t0[:, : DC // 2], wsT.ap()[g, IC, :, : DC // 2])
                nc.scalar.dma_start(
                    v1t0[:, DC // 2 :], wsT.ap()[g, IC, :, DC // 2 :]
                )
                for j in range(1, MG):
                    for k, dc in enumerate(range(0, DC, 2)):
                        eng = nc.scalar if (j + k) % 2 else nc.sync
                        eng.dma_start(
                            xts[j][:, dc : dc + 2], xTs[g].ap()[j, :, dc : dc + 2]
                        )

                # mm1 + SwiGLU: weights outer, token tiles inner (weight reuse)
                for ip in range(IC):
                    if ip == 0:
                        w1t, v1t = w1t0, v1t0
                    else:
                        w1t = wp.tile([128, DC, 128], bf16, tag="ws")
                        nc.sync.dma_start(w1t[:], wsT.ap()[g, ip])
                        v1t = wp.tile([128, DC, 128], bf16, tag="ws")
                        nc.sync.dma_start(v1t[:], wsT.ap()[g, IC + ip])
                    for j in range(MG):
                        ps1 = pp.tile([128, T], f32, tag="h")
                        ps2 = pp.tile([128, T], f32, tag="h")
                        for dc in range(DC):
                            nc.tensor.matmul(
                                ps1[:], w1t[:, dc], xts[j][:, dc],
                                start=(dc == 0), stop=(dc == DC - 1),
                            )
                        for dc in range(DC):
                            nc.tensor.matmul(
                                ps2[:], v1t[:, dc], xts[j][:, dc],
                                start=(dc == 0), stop=(dc == DC - 1),
                            )
                        st = sp.tile([128, T], f32, tag="silu")
                        nc.scalar.activation(
                            st[:], ps1[:], mybir.ActivationFunctionType.Silu
                        )
                        nc.vector.tensor_mul(acts[j][:, ip], st[:], ps2[:])

                # mm2: w2 slabs outer, token tiles inner (weight reuse)
                last_g = g == len(shapes) - 1
                for db in range(DC):
                    w2t = w2p.tile([128, IC, 128], bf16, tag="w2")
                    nc.sync.dma_start(w2t[:], w2T.ap()[g, db])
                    for j in range(MG):
                        # last slab of the kernel: compute/copy/DMA in column
                        # halves (separate PSUM tiles — same tile would add a
                        # false matmul->copy dependency) so the writeback
                        # overlaps the final matmuls
                        halves = (
                            2 if last_g and db == DC - 1 and j == MG - 1 else 1
                        )
                        Th = T // halves
                        for h in range(halves):
                            sl = slice(h * Th, (h + 1) * Th)
                            ps3 = pp2.tile([128, Th], f32, tag="y")
                            for ic in range(IC):
                                nc.tensor.matmul(
                                    ps3[:], w2t[:, ic], acts[j][:, ic, sl],
                                    start=(ic == 0), stop=(ic == IC - 1),
                                )
                            ot = sp.tile([128, Th], bf16, tag="yout")
                            nc.any.tensor_copy(ot[:], ps3[:])
                            eng = nc.scalar if h % 2 else nc.sync
                            eng.dma_start(yTs[g].ap()[j, db, :, sl], ot[:])
    nc.compile()
    return nc


def _routing(x, rw):
    logits = x @ rw.T
    m = logits.max(-1, keepdims=True)
    p = np.exp(logits - m)
    p /= p.sum(-1, keepdims=True)
    topk_idx = np.argpartition(-p, TOP_K - 1, axis=-1)[:, :TOP_K]
    topk_val = np.take_along_axis(p, topk_idx, -1)
    topk_val = topk_val / topk_val.sum(-1, keepdims=True)
    return topk_idx, topk_val


def _group_shape(cmax):
    """Pick (MG, T) so MG*T >= cmax, T in [128, 512] mult of 4 (even halves),
    minimizing MG*T (tie: prefer fewer/larger tiles)."""
    best = None
    for MG in range(1, 17):
        T = -(-cmax // MG) if cmax else 128
        T = (T + 3) // 4 * 4
        if T > 512:
            continue
        T = max(T, 128)
        if best is None or MG * T < best[0]:
            best = (MG * T, MG, T)
    assert best is not None
    return best[1], best[2]


def _tile_ws(ws_e):
    # [cb, p, dc, col] = ws_e[cb*128+col, dc*128+p]
    return np.ascontiguousarray(
        ws_e.reshape(CB, 128, DC, 128).transpose(0, 3, 2, 1)
    )


def _tile_w2(w2_e):
    # [db, p, ic, col] = w2_e[db*128+col, ic*128+p]
    return np.ascontiguousarray(
        w2_e.reshape(DC, 128, IC, 128).transpose(0, 3, 2, 1)
    )


NGROUPS = 4  # expert pieces per core: splits experts for load balance


def _plan(counts, n_groups):
    """Split the E experts' token counts into n_groups*N_CORES near-equal
    pieces (each piece = contiguous token range of one expert), sort pieces
    by size, and pack rank-consecutive pieces into groups of N_CORES slots.
    Returns (shapes, groups): shapes[g] = (MG, T); groups[g][c] =
    (expert, start, size) for core c."""
    nslots = n_groups * N_CORES
    n = np.ones(E, dtype=np.int64)
    while n.sum() < nslots:
        piece = -(-counts // n)
        n[int(np.argmax(piece))] += 1
    pieces = []
    for e in range(E):
        k = int(n[e])
        base, rem = divmod(int(counts[e]), k)
        off = 0
        for i in range(k):
            sz = base + (1 if i < rem else 0)
            pieces.append((sz, e, off))
            off += sz
    pieces.sort(key=lambda p: (-p[0], p[1], p[2]))
    groups = [
        [(e, off, sz) for sz, e, off in pieces[g * N_CORES : (g + 1) * N_CORES]]
        for g in range(n_groups)
    ]
    shapes = tuple(
        _group_shape(max(sz for _, _, sz in grp)) for grp in groups
    )
    return shapes, groups


def kernel(hidden_states, router_w, ws, w2s):
    global LAST_EXEC_NS
    x = np.ascontiguousarray(np.asarray(hidden_states, dtype=np.float32))
    rw = np.asarray(router_w, dtype=np.float32)
    ws = np.asarray(ws, dtype=np.float32)
    w2s = np.asarray(w2s, dtype=np.float32)
    T_tok = x.shape[0]

    topk_idx, topk_val = _routing(x, rw)

    expert_tok = []
    expert_gate = []
    for e in range(E):
        hit = topk_idx == e
        rows = np.nonzero(hit.any(-1))[0]
        gv = np.where(hit[rows], topk_val[rows], 0.0).sum(-1).astype(np.float32)
        expert_tok.append(rows)
        expert_gate.append(gv)

    counts = np.array([len(t) for t in expert_tok])
    shapes, groups = _plan(counts, NGROUPS)
    NG = len(shapes)

    if shapes not in _compiled:
        _compiled[shapes] = _build_program(shapes)
    nc = _compiled[shapes]

    x16 = x.astype(BF16)
    ws16 = {}
    w2s16 = {}
    in_maps = []
    for c in range(N_CORES):
        m = {}
        wsT_b = np.empty((NG, CB, 128, DC, 128), dtype=BF16)
        w2T_b = np.empty((NG, DC, 128, IC, 128), dtype=BF16)
        for g, (MG, T) in enumerate(shapes):
            e, off, sz = groups[g][c]
            if e not in ws16:
                ws16[e] = _tile_ws(ws[e].astype(BF16))
                w2s16[e] = _tile_w2(w2s[e].astype(BF16))
            wsT_b[g] = ws16[e]
            w2T_b[g] = w2s16[e]
            xT_b = np.zeros((MG, 128, DC, T), dtype=BF16)
            toks = expert_tok[e][off : off + sz]
            for j in range(MG):
                seg = toks[j * T : (j + 1) * T]
                nn = len(seg)
                if nn == 0:
                    continue
                xT_b[j, :, :, :nn] = x16[seg].reshape(nn, DC, 128).transpose(2, 1, 0)
            m[f"xT{g}"] = xT_b
        m["wsT"] = wsT_b
        m["w2T"] = w2T_b
        in_maps.append(m)

    res = run_bass_kernel_spmd(
        nc, in_maps, core_ids=list(range(N_CORES)), trace=TRACE
    )
    LAST_EXEC_NS = res.exec_time_ns

    out = np.zeros((T_tok, D), dtype=np.float32)
    for g, (MG, T) in enumerate(shapes):
        for c in range(N_CORES):
            e, off, sz = groups[g][c]
            toks = expert_tok[e][off : off + sz]
            gates = expert_gate[e][off : off + sz]
            yT_c = np.asarray(res.results[c][f"yT{g}"]).astype(np.float32)
            for j in range(MG):
                seg = toks[j * T : (j + 1) * T]
                nn = len(seg)
                if nn == 0:
                    break
                y_item = yT_c[j].transpose(2, 0, 1).reshape(T, D)[:nn]
                out[seg] += gates[j * T : (j + 1) * T][:, None] * y_item
    return out


# revision 13
# speedup vs baseline: 1.0373x; 1.0212x over previous
"""DBRX MoE experts kernel for 8 Trainium2 NeuronCores.

Strategy (expert-parallel with host-side token dispatch + expert splitting):
  - Host computes the (cheap) router: softmax over 16 experts, top-4,
    renormalized gates.  Tokens are gathered per expert.
  - Experts are split into NGROUPS*8 near-equal pieces (each piece a
    contiguous chunk of one expert's token list).  Pieces are sorted by size
    and packed into NGROUPS groups of 8 slots; each core gets one piece per
    group, padded to the group's (MG, T) tile shape.  This balances the load
    across cores to ~2% above ideal.
  - Device (SPMD, one program on all 8 cores) runs the expert FFN:
    h = wsT.T @ x (both halves), act = silu(h1)*h2, y = w2T.T @ act.
    All matmuls in bfloat16 (1 cycle/row, FWL weight loads, half the DMA
    bytes of fp32; end-to-end ~4e-3 rel err vs 2e-2 budget).
  - Weight loads are paired (w1+v1 per ip in one DMA, 2 w2 slabs per DMA)
    and outputs paired (2 D-slabs per DMA) to halve the HWDGE descriptor-
    generation load on the two issue engines (sync/scalar), which otherwise
    run ~95% busy and delay group transitions.
  - Host applies gates and scatter-adds piece outputs into the [T, D] output.

Self-contained: hardcodes T=4096 tokens, D=1024, I=2048, E=16, top_k=4,
8 cores.
"""

import sys

if "/opt/trn_rl_repo" not in sys.path:
    sys.path.insert(0, "/opt/trn_rl_repo")

import ml_dtypes
import numpy as np

import concourse.bacc as bacc
import concourse.mybir as mybir
import concourse.tile as tile
from concourse.bass_utils import run_bass_kernel_spmd

TOP_K = 4
N_CORES = 8
D = 1024
I = 2048
E = 16
DC = D // 128  # 8 contraction chunks for mm1 / output blocks for mm2
IC = I // 128  # 16 intermediate blocks
DP = DC // 2  # mm2 output slab pairs

BF16 = ml_dtypes.bfloat16

NGROUPS = 4  # expert pieces per core: splits experts for load balance

TRACE = False
LAST_EXEC_NS = None

_compiled = {}  # shapes tuple -> nc


def _build_program(shapes):
    """shapes: tuple of (MG, T) per group (one group slot = one expert piece)."""
    bf16 = mybir.dt.bfloat16
    f32 = mybir.dt.float32
    NG = len(shapes)
    nc = bacc.Bacc("TRN2", target_bir_lowering=False, debug=False, num_devices=N_CORES)

    xTs, yTs = [], []
    for g, (MG, T) in enumerate(shapes):
        xTs.append(
            nc.dram_tensor(f"xT{g}", [MG, 128, DC, T], bf16, kind="ExternalInput")
        )
        yTs.append(
            nc.dram_tensor(f"yT{g}", [MG, DP, 128, 2, T], bf16, kind="ExternalOutput")
        )
    # ws pairs (w1 block, v1 block) adjacently so one DMA loads both
    wsT = nc.dram_tensor("wsT", [NG, IC, 128, 2, DC, 128], bf16, kind="ExternalInput")
    # w2 pairs two D-slabs per DMA
    w2T = nc.dram_tensor("w2T", [NG, DP, 128, 2, IC, 128], bf16, kind="ExternalInput")

    with tile.TileContext(nc) as tc:
        with (
            tc.tile_pool(name="xp", bufs=max(mg for mg, _ in shapes) + 1) as xp,
            tc.tile_pool(name="wp", bufs=6) as wp,
            tc.tile_pool(name="w2p", bufs=3) as w2p,
            tc.tile_pool(name="actp", bufs=max(mg for mg, _ in shapes)) as actp,
            tc.tile_pool(name="sp", bufs=3) as sp,
            tc.tile_pool(name="pp", bufs=6, space="PSUM") as pp,
            tc.tile_pool(name="pp2", bufs=2, space="PSUM") as pp2,
        ):
            Tmax = max(t for _, t in shapes)
            heads = {}  # g -> (wt0, xts, acts)

            def emit_head(g):
                """Issue group g's startup DMAs: first weight pair + x tiles.
                Called one group early so the transfers hide behind compute."""
                MG, T = shapes[g]
                wt0 = wp.tile([128, 2, DC, 128], bf16, tag="ws", name=f"wt0_{g}")
                # w1 half first (ps1 needs it first), halves on both engines
                nc.sync.dma_start(wt0[:, 0], wsT.ap()[g, 0, :, 0])
                nc.scalar.dma_start(wt0[:, 1], wsT.ap()[g, 0, :, 1])
                xts, acts = [], []
                for j in range(MG):
                    xt = xp.tile(
                        [128, DC, Tmax], bf16, tag="x", name=f"x_{g}_{j}"
                    )[:, :, :T]
                    xts.append(xt)
                    acts.append(
                        actp.tile(
                            [128, IC, Tmax], bf16, tag="act", name=f"act_{g}_{j}"
                        )[:, :, :T]
                    )
                nch = 4 if g == 0 else 2
                for j in range(MG):
                    step = DC // nch
                    for k, dc in enumerate(range(0, DC, step)):
                        eng = nc.scalar if (j + k) % 2 else nc.sync
                        eng.dma_start(
                            xts[j][:, dc : dc + step],
                            xTs[g].ap()[j, :, dc : dc + step],
                        )
                heads[g] = (wt0, xts, acts)

            emit_head(0)
            for g, (MG, T) in enumerate(shapes):
                wt0, xts, acts = heads[g]

                # mm1 + SwiGLU: weights outer, token tiles inner (weight reuse)
                for ip in range(IC):
                    if ip == 0:
                        wt = wt0
                    else:
                        wt = wp.tile([128, 2, DC, 128], bf16, tag="ws")
                        nc.sync.dma_start(wt[:], wsT.ap()[g, ip])
                    for j in range(MG):
                        ps1 = pp.tile([128, T], f32, tag="h")
                        ps2 = pp.tile([128, T], f32, tag="h")
                        for dc in range(DC):
                            nc.tensor.matmul(
                                ps1[:], wt[:, 0, dc], xts[j][:, dc],
                                start=(dc == 0), stop=(dc == DC - 1),
                            )
                        for dc in range(DC):
                            nc.tensor.matmul(
                                ps2[:], wt[:, 1, dc], xts[j][:, dc],
                                start=(dc == 0), stop=(dc == DC - 1),
                            )
                        st = sp.tile([128, T], f32, tag="silu")
                        nc.scalar.activation(
                            st[:], ps1[:], mybir.ActivationFunctionType.Silu
                        )
                        nc.vector.tensor_mul(acts[j][:, ip], st[:], ps2[:])

                # next group's head DMAs issue before mm2's weight stream so
                # the group transition doesn't stall on the issue queues
                if g + 1 < NG:
                    emit_head(g + 1)

                # mm2: w2 slab pairs outer, token tiles inner (weight reuse)
                last_g = g == NG - 1
                for dbp in range(DP):
                    w2t = w2p.tile([128, 2, IC, 128], bf16, tag="w2")
                    nc.sync.dma_start(w2t[:], w2T.ap()[g, dbp])
                    for j in range(MG):
                        last = last_g and dbp == DP - 1 and j == MG - 1
                        ot = sp.tile([128, 2, T], bf16, tag="yout")
                        for h in range(2):
                            ps3 = pp2.tile([128, T], f32, tag="y")
                            for ic in range(IC):
                                nc.tensor.matmul(
                                    ps3[:], w2t[:, h, ic], acts[j][:, ic],
                                    start=(ic == 0), stop=(ic == IC - 1),
                                )
                            nc.any.tensor_copy(ot[:, h], ps3[:])
                            if last:
                                # final slabs: write each half immediately so
                                # the last DMA chases the last matmul
                                eng = nc.scalar if h else nc.sync
                                eng.dma_start(yTs[g].ap()[j, dbp, :, h], ot[:, h])
                        if not last:
                            nc.sync.dma_start(yTs[g].ap()[j, dbp], ot[:])
    nc.compile()
    return nc


def _routing(x, rw):
    logits = x @ rw.T
    m = logits.max(-1, keepdims=True)
    p = np.exp(logits - m)
    p /= p.sum(-1, keepdims=True)
    topk_idx = np.argpartition(-p, TOP_K - 1, axis=-1)[:, :TOP_K]
    topk_val = np.take_along_axis(p, topk_idx, -1)
    topk_val = topk_val / topk_val.sum(-1, keepdims=True)
    return topk_idx, topk_val


def _group_shape(cmax):
    """Pick (MG, T) so MG*T >= cmax, T in [128, 512] mult of 4, minimizing
    MG*T."""
    best = None
    for MG in range(1, 17):
        T = -(-cmax // MG) if cmax else 128
        T = (T + 3) // 4 * 4
        if T > 512:
            continue
        T = max(T, 128)
        if best is None or MG * T < best[0]:
            best = (MG * T, MG, T)
    assert best is not None
    return best[1], best[2]


def _plan(counts, n_groups):
    """Split the E experts' token counts into n_groups*N_CORES near-equal
    pieces (each piece = contiguous token range of one expert), sort pieces
    by size, and pack rank-consecutive pieces into groups of N_CORES slots.
    Returns (shapes, groups): shapes[g] = (MG, T); groups[g][c] =
    (expert, start, size) for core c."""
    nslots = n_groups * N_CORES
    n = np.ones(E, dtype=np.int64)
    while n.sum() < nslots:
        piece = -(-counts // n)
        n[int(np.argmax(piece))] += 1
    pieces = []
    for e in range(E):
        k = int(n[e])
        base, rem = divmod(int(counts[e]), k)
        off = 0
        for i in range(k):
            sz = base + (1 if i < rem else 0)
            pieces.append((sz, e, off))
            off += sz
    pieces.sort(key=lambda p: (-p[0], p[1], p[2]))
    groups = [
        [(e, off, sz) for sz, e, off in pieces[g * N_CORES : (g + 1) * N_CORES]]
        for g in range(n_groups)
    ]
    shapes = tuple(
        _group_shape(max(sz for _, _, sz in grp)) for grp in groups
    )
    return shapes, groups


def _tile_ws(ws_e):
    # [ip, p, h, dc, col] = ws_e[h*I + ip*128 + col, dc*128 + p]
    return np.ascontiguousarray(
        ws_e.reshape(2, IC, 128, DC, 128).transpose(1, 4, 0, 3, 2)
    )


def _tile_w2(w2_e):
    # [dbp, p, h, ic, col] = w2_e[(2*dbp+h)*128 + col, ic*128 + p]
    return np.ascontiguousarray(
        w2_e.reshape(DP, 2, 128, IC, 128).transpose(0, 4, 1, 3, 2)
    )


def kernel(hidden_states, router_w, ws, w2s):
    global LAST_EXEC_NS
    x = np.ascontiguousarray(np.asarray(hidden_states, dtype=np.float32))
    rw = np.asarray(router_w, dtype=np.float32)
    ws = np.asarray(ws, dtype=np.float32)
    w2s = np.asarray(w2s, dtype=np.float32)
    T_tok = x.shape[0]

    topk_idx, topk_val = _routing(x, rw)

    expert_tok = []
    expert_gate = []
    for e in range(E):
        hit = topk_idx == e
        rows = np.nonzero(hit.any(-1))[0]
        gv = np.where(hit[rows], topk_val[rows], 0.0).sum(-1).astype(np.float32)
        expert_tok.append(rows)
        expert_gate.append(gv)

    counts = np.array([len(t) for t in expert_tok])
    shapes, groups = _plan(counts, NGROUPS)
    NG = len(shapes)

    if shapes not in _compiled:
        _compiled[shapes] = _build_program(shapes)
    nc = _compiled[shapes]

    x16 = x.astype(BF16)
    ws16 = {}
    w2s16 = {}
    in_maps = []
    for c in range(N_CORES):
        m = {}
        wsT_b = np.empty((NG, IC, 128, 2, DC, 128), dtype=BF16)
        w2T_b = np.empty((NG, DP, 128, 2, IC, 128), dtype=BF16)
        for g, (MG, T) in enumerate(shapes):
            e, off, sz = groups[g][c]
            if e not in ws16:
                ws16[e] = _tile_ws(ws[e].astype(BF16))
                w2s16[e] = _tile_w2(w2s[e].astype(BF16))
            wsT_b[g] = ws16[e]
            w2T_b[g] = w2s16[e]
            xT_b = np.zeros((MG, 128, DC, T), dtype=BF16)
            toks = expert_tok[e][off : off + sz]
            for j in range(MG):
                seg = toks[j * T : (j + 1) * T]
                nn = len(seg)
                if nn == 0:
                    continue
                xT_b[j, :, :, :nn] = x16[seg].reshape(nn, DC, 128).transpose(2, 1, 0)
            m[f"xT{g}"] = xT_b
        m["wsT"] = wsT_b
        m["w2T"] = w2T_b
        in_maps.append(m)

    res = run_bass_kernel_spmd(
        nc, in_maps, core_ids=list(range(N_CORES)), trace=TRACE
    )
    LAST_EXEC_NS = res.exec_time_ns

    out = np.zeros((T_tok, D), dtype=np.float32)
    for g, (MG, T) in enumerate(shapes):
        for c in range(N_CORES):
            e, off, sz = groups[g][c]
            toks = expert_tok[e][off : off + sz]
            gates = expert_gate[e][off : off + sz]
            yT_c = np.asarray(res.results[c][f"yT{g}"]).astype(np.float32)
            for j in range(MG):
                seg = toks[j * T : (j + 1) * T]
                nn = len(seg)
                if nn == 0:
                    break
                # [dbp, p, h, t] -> [t, dbp, h, p] -> [T, D]
                y_item = yT_c[j].transpose(3, 0, 2, 1).reshape(T, D)[:nn]
                out[seg] += gates[j * T : (j + 1) * T][:, None] * y_item
    return out


# revision 15
# speedup vs baseline: 1.0582x; 1.0201x over previous
"""DBRX MoE experts kernel for 8 Trainium2 NeuronCores.

Strategy (expert-parallel with host-side token dispatch + expert splitting):
  - Host computes the (cheap) router: softmax over 16 experts, top-4,
    renormalized gates.  Tokens are gathered per expert.
  - Experts are split into NGROUPS*8 near-equal pieces (each piece a
    contiguous chunk of one expert's token list).  Pieces are sorted by size
    and packed into NGROUPS groups of 8 slots; each core gets one piece per
    group, padded to the group's (MG, T) tile shape.  This balances the load
    across cores to ~2% above ideal.
  - Device (SPMD, one program on all 8 cores) runs the expert FFN:
    h = wsT.T @ x (both halves), act = silu(h1)*h2, y = w2T.T @ act.
    All matmuls in bfloat16 (1 cycle/row, FWL weight loads, half the DMA
    bytes of fp32; end-to-end ~4e-3 rel err vs 2e-2 budget).
  - Weight loads are paired (w1+v1 per ip in one DMA, 2 w2 slabs per DMA)
    and outputs paired (2 D-slabs per DMA) to halve the HWDGE descriptor-
    generation load on the two issue engines (sync/scalar), which otherwise
    run ~95% busy and delay group transitions.
  - Host applies gates and scatter-adds piece outputs into the [T, D] output.

Self-contained: hardcodes T=4096 tokens, D=1024, I=2048, E=16, top_k=4,
8 cores.
"""

import sys

if "/opt/trn_rl_repo" not in sys.path:
    sys.path.insert(0, "/opt/trn_rl_repo")

import ml_dtypes
import numpy as np

import concourse.bacc as bacc
import concourse.mybir as mybir
import concourse.tile as tile
from concourse.bass_utils import run_bass_kernel_spmd

TOP_K = 4
N_CORES = 8
D = 1024
I = 2048
E = 16
DC = D // 128  # 8 contraction chunks for mm1 / output blocks for mm2
IC = I // 128  # 16 intermediate blocks
DP = DC // 2  # mm2 output slab pairs

BF16 = ml_dtypes.bfloat16

NGROUPS = 4  # expert pieces per core: splits experts for load balance

TRACE = False
LAST_EXEC_NS = None

_compiled = {}  # shapes tuple -> nc


def _build_program(shapes):
    """shapes: tuple of (MG, T) per group (one group slot = one expert piece)."""
    bf16 = mybir.dt.bfloat16
    f32 = mybir.dt.float32
    NG = len(shapes)
    nc = bacc.Bacc("TRN2", target_bir_lowering=False, debug=False, num_devices=N_CORES)

    xTs, yTs = [], []
    for g, (MG, T) in enumerate(shapes):
        xTs.append(
            nc.dram_tensor(f"xT{g}", [MG, 128, DC, T], bf16, kind="ExternalInput")
        )
        yTs.append(
            nc.dram_tensor(f"yT{g}", [MG, DP, 128, 2, T], bf16, kind="ExternalOutput")
        )
    # ws pairs (w1 block, v1 block) adjacently so one DMA loads both
    wsT = nc.dram_tensor("wsT", [NG, IC, 128, 2, DC, 128], bf16, kind="ExternalInput")
    # w2 pairs two D-slabs per DMA
    w2T = nc.dram_tensor("w2T", [NG, DP, 128, 2, IC, 128], bf16, kind="ExternalInput")

    with tile.TileContext(nc) as tc:
        with (
            tc.tile_pool(name="xp", bufs=max(mg for mg, _ in shapes) + 1) as xp,
            tc.tile_pool(name="wp", bufs=6) as wp,
            tc.tile_pool(name="w2p", bufs=3) as w2p,
            tc.tile_pool(name="actp", bufs=max(mg for mg, _ in shapes)) as actp,
            tc.tile_pool(name="sp", bufs=3) as sp,
            tc.tile_pool(name="pp", bufs=6, space="PSUM") as pp,
            tc.tile_pool(name="pp2", bufs=2, space="PSUM") as pp2,
        ):
            Tmax = max(t for _, t in shapes)
            heads = {}  # g -> (wt0, xts, acts)

            def emit_head(g):
                """Issue group g's startup DMAs: first weight pair + x tiles.
                Called one group early so the transfers hide behind compute."""
                MG, T = shapes[g]
                wt0 = wp.tile([128, 2, DC, 128], bf16, tag="ws", name=f"wt0_{g}")
                # w1 half first (ps1 needs it first), halves on both engines
                nc.sync.dma_start(wt0[:, 0], wsT.ap()[g, 0, :, 0])
                nc.scalar.dma_start(wt0[:, 1], wsT.ap()[g, 0, :, 1])
                xts, acts = [], []
                for j in range(MG):
                    xt = xp.tile(
                        [128, DC, Tmax], bf16, tag="x", name=f"x_{g}_{j}"
                    )[:, :, :T]
                    xts.append(xt)
                    acts.append(
                        actp.tile(
                            [128, IC, Tmax], bf16, tag="act", name=f"act_{g}_{j}"
                        )[:, :, :T]
                    )
                nch = 4 if g == 0 else 2
                for j in range(MG):
                    step = DC // nch
                    for k, dc in enumerate(range(0, DC, step)):
                        eng = nc.scalar if (j + k) % 2 else nc.sync
                        eng.dma_start(
                            xts[j][:, dc : dc + step],
                            xTs[g].ap()[j, :, dc : dc + step],
                        )
                heads[g] = (wt0, xts, acts)

            emit_head(0)
            for g, (MG, T) in enumerate(shapes):
                wt0, xts, acts = heads[g]

                # mm1 + SwiGLU: weights outer, token tiles inner (weight reuse)
                for ip in range(IC):
                    if ip == 0:
                        wt = wt0
                    else:
                        wt = wp.tile([128, 2, DC, 128], bf16, tag="ws")
                        nc.sync.dma_start(wt[:], wsT.ap()[g, ip])
                    for j in range(MG):
                        ps1 = pp.tile([128, T], f32, tag="h")
                        ps2 = pp.tile([128, T], f32, tag="h")
                        for dc in range(DC):
                            nc.tensor.matmul(
                                ps1[:], wt[:, 0, dc], xts[j][:, dc],
                                start=(dc == 0), stop=(dc == DC - 1),
                            )
                        for dc in range(DC):
                            nc.tensor.matmul(
                                ps2[:], wt[:, 1, dc], xts[j][:, dc],
                                start=(dc == 0), stop=(dc == DC - 1),
                            )
                        st = sp.tile([128, T], f32, tag="silu")
                        nc.scalar.activation(
                            st[:], ps1[:], mybir.ActivationFunctionType.Silu
                        )
                        nc.vector.tensor_mul(acts[j][:, ip], st[:], ps2[:])

                # next group's head DMAs issue before mm2's weight stream so
                # the group transition doesn't stall on the issue queues
                if g + 1 < NG:
                    emit_head(g + 1)

                # mm2: w2 slab pairs outer, token tiles inner (weight reuse)
                last_g = g == NG - 1
                for dbp in range(DP):
                    w2t = w2p.tile([128, 2, IC, 128], bf16, tag="w2")
                    nc.sync.dma_start(w2t[:], w2T.ap()[g, dbp])
                    for j in range(MG):
                        last = last_g and dbp == DP - 1 and j == MG - 1
                        ot = sp.tile([128, 2, T], bf16, tag="yout")
                        for h in range(2):
                            ps3 = pp2.tile([128, T], f32, tag="y")
                            for ic in range(IC):
                                nc.tensor.matmul(
                                    ps3[:], w2t[:, h, ic], acts[j][:, ic],
                                    start=(ic == 0), stop=(ic == IC - 1),
                                )
                            nc.any.tensor_copy(ot[:, h], ps3[:])
                            if last:
                                # final slabs: write each half immediately so
                                # the last DMA chases the last matmul
                                eng = nc.scalar if h else nc.sync
                                eng.dma_start(yTs[g].ap()[j, dbp, :, h], ot[:, h])
                        if not last:
                            nc.sync.dma_start(yTs[g].ap()[j, dbp], ot[:])
    nc.compile()
    return nc


def _routing(x, rw):
    logits = x @ rw.T
    m = logits.max(-1, keepdims=True)
    p = np.exp(logits - m)
    p /= p.sum(-1, keepdims=True)
    topk_idx = np.argpartition(-p, TOP_K - 1, axis=-1)[:, :TOP_K]
    topk_val = np.take_along_axis(p, topk_idx, -1)
    topk_val = topk_val / topk_val.sum(-1, keepdims=True)
    return topk_idx, topk_val


def _group_shape(cmax):
    """Pick (MG, T) so MG*T >= cmax, T in [128, 512] mult of 4, minimizing
    MG*T."""
    best = None
    for MG in range(1, 17):
        T = -(-cmax // MG) if cmax else 128
        T = (T + 3) // 4 * 4
        if T > 512:
            continue
        T = max(T, 128)
        if best is None or MG * T < best[0]:
            best = (MG * T, MG, T)
    assert best is not None
    return best[1], best[2]


def _plan_greedy(counts, n_groups):
    """Split the E experts' token counts into n_groups*N_CORES near-equal
    pieces (each piece = contiguous token range of one expert), sort pieces
    by size, and pack rank-consecutive pieces into groups of N_CORES slots.
    Returns (shapes, groups): shapes[g] = (MG, T); groups[g][c] =
    (expert, start, size) for core c."""
    nslots = n_groups * N_CORES
    n = np.ones(E, dtype=np.int64)
    while n.sum() < nslots:
        piece = -(-counts // n)
        n[int(np.argmax(piece))] += 1
    pieces = []
    for e in range(E):
        k = int(n[e])
        base, rem = divmod(int(counts[e]), k)
        off = 0
        for i in range(k):
            sz = base + (1 if i < rem else 0)
            pieces.append((sz, e, off))
            off += sz
    pieces.sort(key=lambda p: (-p[0], p[1], p[2]))
    groups = [
        [(e, off, sz) for sz, e, off in pieces[g * N_CORES : (g + 1) * N_CORES]]
        for g in range(n_groups)
    ]
    shapes = tuple(
        _group_shape(max(sz for _, _, sz in grp)) for grp in groups
    )
    return shapes, groups


def _plan_optimal(counts, n_groups):
    """Optimal 2-slots-per-expert plan (n_groups=4, E=2*N_CORES only):
    choose 4 group capacities (8 slots each) and a matching expert -> slot
    pair with cap_a+cap_b >= count, minimizing total capacity.  Forward
    reachable-set DP over remaining-slot multisets, experts in descending
    count order.  Returns (shapes, groups) like _plan_greedy, or None."""
    if n_groups != 4 or len(counts) != 2 * N_CORES:
        return None
    order = sorted(range(E), key=lambda e: -counts[e])
    cs = [int(counts[e]) for e in order]
    total = sum(cs)
    # achievable slot capacities: MG=1: T<=512 mult4; MG=2: mult8 <= 1024.
    # Window: caps near half-expert size (2-split structures); whole-expert
    # slots only matter for worse-than-greedy totals, which the greedy
    # fallback covers.
    ach = sorted(
        {t for t in range(128, 513, 4)} | {2 * t for t in range(128, 513, 4)}
    )
    lo = max(128, cs[-1] // 2 - 64)
    hi = min(1024, cs[0] // 2 + 192)
    vals = sorted([v for v in ach if lo <= v <= hi], reverse=True)
    greedy_cap = sum(
        mg * t for mg, t in _plan_greedy(counts, n_groups)[0]
    )
    smin = -(-total // N_CORES)
    smin = (smin + 3) // 4 * 4

    def combos_for_total(S):
        out = []

        def rec(k, rem, start, acc):
            if k == 0:
                if rem == 0:
                    out.append(tuple(acc))
                return
            for i in range(start, len(vals)):
                v = vals[i]
                if v * k < rem:
                    break
                if rem - v > (k - 1) * v:
                    continue
                if rem - v < 0:
                    continue
                acc.append(v)
                rec(k - 1, rem - v, i, acc)
                acc.pop()

        rec(4, S, 0, [])
        return out

    cands = []
    for S in range(smin, greedy_cap, 4):
        for caps in combos_for_total(S):
            cands.append((S, caps))
    pairs = [(a, b) for a in range(4) for b in range(a, 4)]
    for tot, caps in cands:
        # reachable-set DP with parent pointers for backtracking
        states = {(8, 8, 8, 8): None}
        layers = [states]
        ok = True
        for i, c in enumerate(cs):
            nxt = {}
            for rem in layers[-1]:
                for a, b in pairs:
                    if a == b:
                        if rem[a] < 2:
                            continue
                    elif rem[a] == 0 or rem[b] == 0:
                        continue
                    if caps[a] + caps[b] < c:
                        continue
                    r2 = list(rem)
                    r2[a] -= 1
                    r2[b] -= 1
                    r2 = tuple(r2)
                    if r2 not in nxt:
                        nxt[r2] = (rem, a, b)
            if not nxt:
                ok = False
                break
            layers.append(nxt)
        if not ok:
            continue
        # backtrack
        state = next(iter(layers[-1]))
        assign = [None] * E
        for i in range(E - 1, -1, -1):
            prev, a, b = layers[i + 1][state]
            assign[i] = (a, b)
            state = prev
        # build groups: cut each expert to fill slot a fully, rest in slot b
        groups = [[] for _ in range(4)]
        for i, e in enumerate(order):
            a, b = assign[i]
            c = cs[i]
            pa = min(caps[a], c)
            groups[a].append((e, 0, pa))
            groups[b].append((e, pa, c - pa))
        for g in range(4):
            assert len(groups[g]) == N_CORES
        shapes = tuple(_group_shape(caps[g]) for g in range(4))
        return shapes, groups
    return None


def _plan(counts, n_groups):
    plan = _plan_optimal(counts, n_groups)
    if plan is not None:
        return plan
    return _plan_greedy(counts, n_groups)


def _tile_ws(ws_e):
    # [ip, p, h, dc, col] = ws_e[h*I + ip*128 + col, dc*128 + p]
    return np.ascontiguousarray(
        ws_e.reshape(2, IC, 128, DC, 128).transpose(1, 4, 0, 3, 2)
    )


def _tile_w2(w2_e):
    # [dbp, p, h, ic, col] = w2_e[(2*dbp+h)*128 + col, ic*128 + p]
    return np.ascontiguousarray(
        w2_e.reshape(DP, 2, 128, IC, 128).transpose(0, 4, 1, 3, 2)
    )


def kernel(hidden_states, router_w, ws, w2s):
    global LAST_EXEC_NS
    x = np.ascontiguousarray(np.asarray(hidden_states, dtype=np.float32))
    rw = np.asarray(router_w, dtype=np.float32)
    ws = np.asarray(ws, dtype=np.float32)
    w2s = np.asarray(w2s, dtype=np.float32)
    T_tok = x.shape[0]

    topk_idx, topk_val = _routing(x, rw)

    expert_tok = []
    expert_gate = []
    for e in range(E):
        hit = topk_idx == e
        rows = np.nonzero(hit.any(-1))[0]
        gv = np.where(hit[rows], topk_val[rows], 0.0).sum(-1).astype(np.float32)
        expert_tok.append(rows)
        expert_gate.append(gv)

    counts = np.array([len(t) for t in expert_tok])
    shapes, groups = _plan(counts, NGROUPS)
    NG = len(shapes)

    if shapes not in _compiled:
        _compiled[shapes] = _build_program(shapes)
    nc = _compiled[shapes]

    x16 = x.astype(BF16)
    ws16 = {}
    w2s16 = {}
    in_maps = []
    for c in range(N_CORES):
        m = {}
        wsT_b = np.empty((NG, IC, 128, 2, DC, 128), dtype=BF16)
        w2T_b = np.empty((NG, DP, 128, 2, IC, 128), dtype=BF16)
        for g, (MG, T) in enumerate(shapes):
            e, off, sz = groups[g][c]
            if e not in ws16:
                ws16[e] = _tile_ws(ws[e].astype(BF16))
                w2s16[e] = _tile_w2(w2s[e].astype(BF16))
            wsT_b[g] = ws16[e]
            w2T_b[g] = w2s16[e]
            xT_b = np.zeros((MG, 128, DC, T), dtype=BF16)
            toks = expert_tok[e][off : off + sz]
            for j in range(MG):
                seg = toks[j * T : (j + 1) * T]
                nn = len(seg)
                if nn == 0:
                    continue
                xT_b[j, :, :, :nn] = x16[seg].reshape(nn, DC, 128).transpose(2, 1, 0)
            m[f"xT{g}"] = xT_b
        m["wsT"] = wsT_b
        m["w2T"] = w2T_b
        in_maps.append(m)

    res = run_bass_kernel_spmd(
        nc, in_maps, core_ids=list(range(N_CORES)), trace=TRACE
    )
    LAST_EXEC_NS = res.exec_time_ns

    out = np.zeros((T_tok, D), dtype=np.float32)
    for g, (MG, T) in enumerate(shapes):
        for c in range(N_CORES):
            e, off, sz = groups[g][c]
            toks = expert_tok[e][off : off + sz]
            gates = expert_gate[e][off : off + sz]
            yT_c = np.asarray(res.results[c][f"yT{g}"]).astype(np.float32)
            for j in range(MG):
                seg = toks[j * T : (j + 1) * T]
                nn = len(seg)
                if nn == 0:
                    break
                # [dbp, p, h, t] -> [t, dbp, h, p] -> [T, D]
                y_item = yT_c[j].transpose(3, 0, 2, 1).reshape(T, D)[:nn]
                out[seg] += gates[j * T : (j + 1) * T][:, None] * y_item
    return out
